# revision 12
# baseline (speedup 1.0000x reference)
"""Multi-head self-attention (ANE-style 1x1-conv attention) on 8 trn2 cores.

Sharding: zero-communication split over (batch, L-half). Core c handles
batch b = c//2 and query positions [half*1024, half*1024+1024) where
half = c%2. Each core computes k/v over the full L (keys/values are
needed for every query), so k/v projection work is duplicated 2x --
the price of avoiding any cross-core collective.

Per-core pipeline (all matmuls float32r = 1-pass FP22, full PE rate):
  1. q = wq @ x_half      -> q_spill (DRAM)   [o-major]
  2. k = wk @ x           -> k_spill (DRAM)   [o-major]
  3. vT = xT @ wvT        -> v_spill (DRAM)   [L-major, fp16]
  4. per head-pair: sT_chunk = k_h^T q_h (j on partitions, i free),
     p = exp(sT/8) via ACT (the bottleneck engine), O += vaug^T p where
     vaug has a ones column so row 64 of O accumulates the softmax
     denominator. Normalize O rows by 1/denom (DMA-broadcast + DVE mul).
  5. yT = O^T-proj: lhsT=O chunks, rhs=woT  (+ bias, DVE) -> yT output.

Host gathers: out[b, :, 0, half] = yT.T per core.
"""

import numpy as np

import concourse.bass as bass
import concourse.tile as tile
from concourse import bacc, mybir
from concourse.bass_utils import run_bass_kernel_spmd

B, D, L, H, Dh = 4, 1024, 2048, 16, 64
LH = L // 2  # per-core query range
NCORES = 8
FPR = mybir.dt.float32r
F32 = mybir.dt.float32
F16 = mybir.dt.float16
ACT_EXP = mybir.ActivationFunctionType.Exp
INV_SCALE = 1.0 / 8.0  # 1/sqrt(Dh)

NP = D // 128   # 8 partition-chunks of the model dim
NJC = L // 128  # 16 key chunks per head
NPAIR = H // 2  # 8 head pairs


def _bcast_ap(src: bass.AP, npart: int) -> bass.AP:
    """Broadcast a single-partition AP across npart partitions (DMA only)."""
    return bass.AP(
        tensor=src.tensor,
        offset=src.offset,
        ap=[[0, npart]] + [list(p) for p in src.ap[1:]],
    )


def build_nc():
    nc = bacc.Bacc()
    x = nc.dram_tensor("x", [D, L], FPR, kind="ExternalInput")
    xq = nc.dram_tensor("xq", [D, LH], FPR, kind="ExternalInput")
    wqT = nc.dram_tensor("wqT", [D, D], FPR, kind="ExternalInput")
    wkT = nc.dram_tensor("wkT", [D, D], FPR, kind="ExternalInput")
    wvT = nc.dram_tensor("wvT", [D, D], FPR, kind="ExternalInput")
    woT = nc.dram_tensor("woT", [D, D], FPR, kind="ExternalInput")
    bo = nc.dram_tensor("bo", [1, D], FPR, kind="ExternalInput")
    ones32 = nc.dram_tensor("ones32", [1, 64], FPR, kind="ExternalInput")
    ones16 = nc.dram_tensor("ones16", [1, NJC], F16, kind="ExternalInput")
    yT = nc.dram_tensor("yT", [LH, D], F32, kind="ExternalOutput")

    with tile.TileContext(nc) as tc:
        with (
            nc.allow_low_precision(
                reason="attention probs/values intentionally fp16"),
            tc.tile_pool(name="dram", bufs=1, space="DRAM") as dram,
            tc.tile_pool(name="keep", bufs=1) as keep,
            tc.tile_pool(name="ps", bufs=4, space="PSUM") as ps,
        ):
            q_spill = dram.tile([D, LH], FPR)
            k_spill = dram.tile([D, L], FPR)
            v_spill = dram.tile([L, D], F16)
            o_spill = dram.tile([D, LH], FPR)

            bo_sb = keep.tile([128, D], FPR)
            nc.gpsimd.dma_start(out=bo_sb, in_=_bcast_ap(bo[0:1, :], 128))
            ones_t = keep.tile([1, 64], FPR)
            nc.sync.dma_start(out=ones_t, in_=ones32[0:1, :])

            # ---------------- projections ----------------
            with tc.tile_pool(name="proj", bufs=1) as proj:
                xk = []
                for t in range(NP):
                    xt = proj.tile([128, L], FPR, name=f"xk{t}", tag=f"xk{t}")
                    nc.sync.dma_start(out=xt, in_=x[128 * t:128 * (t + 1), :])
                    xk.append(xt)
                xqk = []
                for t in range(NP):
                    xt = proj.tile([128, LH], FPR, name=f"xq{t}", tag=f"xq{t}")
                    nc.sync.dma_start(out=xt, in_=xq[128 * t:128 * (t + 1), :])
                    xqk.append(xt)

                # q projection: out (o, i) -> q_spill
                for mo in range(NP):
                    wq_t = []
                    for kc in range(NP):
                        wt = proj.tile([128, 128], FPR, name="wq_t",
                                       tag="wq", bufs=18)
                        nc.sync.dma_start(
                            out=wt,
                            in_=wqT[128 * kc:128 * (kc + 1),
                                    128 * mo:128 * (mo + 1)])
                        wq_t.append(wt)
                    for n in range(LH // 512):
                        q_ps = ps.tile([128, 1024], F32, name="pst", tag="pst")
                        for kc in range(NP):
                            nc.tensor.matmul(
                                q_ps[:, 0:512], lhsT=wq_t[kc],
                                rhs=xqk[kc][:, 512 * n:512 * (n + 1)],
                                start=(kc == 0), stop=(kc == NP - 1))
                        qsb = proj.tile([128, 512], FPR, name="qsb",
                                        tag="qsb", bufs=3)
                        nc.vector.tensor_copy(out=qsb, in_=q_ps[:, 0:512])
                        nc.sync.dma_start(
                            out=q_spill[128 * mo:128 * (mo + 1),
                                        512 * n:512 * (n + 1)],
                            in_=qsb)

                # k projection: out (o, l) -> k_spill
                for mo in range(NP):
                    wk_t = []
                    for kc in range(NP):
                        wt = proj.tile([128, 128], FPR, name="wk_t",
                                       tag="wk", bufs=18)
                        nc.sync.dma_start(
                            out=wt,
                            in_=wkT[128 * kc:128 * (kc + 1),
                                    128 * mo:128 * (mo + 1)])
                        wk_t.append(wt)
                    for n in range(L // 512):
                        k_ps = ps.tile([128, 1024], F32, name="pst", tag="pst")
                        for kc in range(NP):
                            nc.tensor.matmul(
                                k_ps[:, 0:512], lhsT=wk_t[kc],
                                rhs=xk[kc][:, 512 * n:512 * (n + 1)],
                                start=(kc == 0), stop=(kc == NP - 1))
                        ksb = proj.tile([128, 512], FPR, name="ksb",
                                        tag="ksb", bufs=3)
                        nc.vector.tensor_copy(out=ksb, in_=k_ps[:, 0:512])
                        nc.sync.dma_start(
                            out=k_spill[128 * mo:128 * (mo + 1),
                                        512 * n:512 * (n + 1)],
                            in_=ksb)

                # vT projection: out (l, o) -> v_spill (fp16)
                wv_t = []
                for kc in range(NP):
                    wt = proj.tile([128, D], FPR, name="wv_t", tag="wv",
                                   bufs=NP)
                    nc.sync.dma_start(out=wt,
                                      in_=wvT[128 * kc:128 * (kc + 1), :])
                    wv_t.append(wt)
                for ml in range(L // 128):
                    v_ps = [
                        ps.tile([128, 1024], F32, name="pst", tag="pst")
                        for _ in range(2)
                    ]
                    for kc in range(NP):
                        for n in range(2):
                            nc.tensor.matmul(
                                v_ps[n][:, 0:512],
                                lhsT=xk[kc][:, 128 * ml:128 * (ml + 1)],
                                rhs=wv_t[kc][:, 512 * n:512 * (n + 1)],
                                start=(kc == 0), stop=(kc == NP - 1))
                    for n in range(2):
                        vsb = proj.tile([128, 512], F16, name="vsb",
                                        tag="vsb", bufs=4)
                        nc.vector.tensor_copy(out=vsb, in_=v_ps[n][:, 0:512])
                        nc.sync.dma_start(
                            out=v_spill[128 * ml:128 * (ml + 1),
                                        512 * n:512 * (n + 1)],
                            in_=vsb)

            # ---------------- attention ----------------
            v_sp_r = v_spill.rearrange("(jc p) o -> p jc o", p=128)
            with tc.tile_pool(name="attn", bufs=1) as attn:
                for t in range(NPAIR):
                    he, ho = 2 * t, 2 * t + 1
                    k_pair = attn.tile([128, L], FPR, name="k_pair",
                                       tag="kp", bufs=2)
                    nc.sync.dma_start(out=k_pair,
                                      in_=k_spill[128 * t:128 * (t + 1), :])
                    q_pair = attn.tile([128, LH], FPR, name="q_pair",
                                       tag="qp", bufs=2)
                    nc.sync.dma_start(out=q_pair,
                                      in_=q_spill[128 * t:128 * (t + 1), :])
                    vaug = attn.tile([128, NJC, 130], F16, name="vaug",
                                     tag="vaug", bufs=2)
                    nc.sync.dma_start(
                        out=vaug[:, :, 0:64],
                        in_=v_sp_r[:, :, 64 * he:64 * (he + 1)])
                    nc.sync.dma_start(
                        out=vaug[:, :, 65:129],
                        in_=v_sp_r[:, :, 64 * ho:64 * (ho + 1)])
                    ones16_b = bass.AP(
                        tensor=ones16, offset=0,
                        ap=[[0, 128], [1, NJC], [1, 1]])
                    nc.sync.dma_start(out=vaug[:, :, 64:65], in_=ones16_b)
                    nc.sync.dma_start(out=vaug[:, :, 129:130], in_=ones16_b)

                    o_ps = [
                        ps.tile([128, 1024], F32, name="pst", tag="pst")
                        for _ in range(2)
                    ]
                    for jc in range(NJC):
                        s_ps = [
                            ps.tile([128, 1024], F32, name="pst", tag="pst")
                            for _ in range(2)
                        ]
                        for e in range(2):
                            lhsT = k_pair[64 * e:64 * (e + 1),
                                          128 * jc:128 * (jc + 1)]
                            for n in range(2):
                                nc.tensor.matmul(
                                    s_ps[e][:, 512 * n:512 * (n + 1)],
                                    lhsT=lhsT,
                                    rhs=q_pair[64 * e:64 * (e + 1),
                                               512 * n:512 * (n + 1)],
                                    start=True, stop=True)
                        for e in range(2):
                            pt = attn.tile([128, LH], F16, name="pt",
                                           tag="pt", bufs=4)
                            nc.scalar.activation(pt, s_ps[e], ACT_EXP,
                                                 scale=INV_SCALE)
                            for n in range(2):
                                nc.tensor.matmul(
                                    o_ps[e][0:65, 512 * n:512 * (n + 1)],
                                    lhsT=vaug[:, jc, 65 * e:65 * (e + 1)],
                                    rhs=pt[:, 512 * n:512 * (n + 1)],
                                    start=(jc == 0), stop=(jc == NJC - 1),
                                    skip_group_check=True)

                    # normalize: rows 0..63 of o_ps[e] / row 64 (denominator)
                    osb = attn.tile([128, LH], FPR, name="osb", tag="osb",
                                    bufs=2)
                    for e in range(2):
                        rcp = attn.tile([128, LH], FPR, name="rcp", tag="rcp",
                                        bufs=2)
                        nc.vector.reciprocal(out=rcp[0:1, :],
                                             in_=o_ps[e][64:65, :])
                        # broadcast 1/denom across 64 partitions via a K=1
                        # matmul against a ones column
                        rb_ps = ps.tile([128, 1024], F32, name="pst",
                                        tag="pst")
                        for n in range(2):
                            nc.tensor.matmul(
                                rb_ps[0:64, 512 * n:512 * (n + 1)],
                                lhsT=ones_t,
                                rhs=rcp[0:1, 512 * n:512 * (n + 1)],
                                start=True, stop=True)
                        rb = attn.tile([64, LH], F32, name="rb", tag="rb",
                                       bufs=2)
                        nc.vector.tensor_copy(out=rb, in_=rb_ps[0:64, :])
                        nc.vector.tensor_mul(
                            out=osb[64 * e:64 * (e + 1), :],
                            in0=o_ps[e][0:64, :], in1=rb)
                    nc.sync.dma_start(out=o_spill[128 * t:128 * (t + 1), :],
                                      in_=osb)

            # ---------------- output projection ----------------
            with tc.tile_pool(name="oproj", bufs=1) as oproj:
                wo_t = []
                for kc in range(NP):
                    wt = oproj.tile([128, D], FPR, name="wo_t", tag="wo",
                                    bufs=NP)
                    nc.sync.dma_start(out=wt,
                                      in_=woT[128 * kc:128 * (kc + 1), :])
                    wo_t.append(wt)
                for mi in range(LH // 128):
                    ot = []
                    for kc in range(NP):
                        t_ = oproj.tile([128, 128], FPR, name="ot", tag="ot",
                                        bufs=18)
                        nc.sync.dma_start(
                            out=t_,
                            in_=o_spill[128 * kc:128 * (kc + 1),
                                        128 * mi:128 * (mi + 1)])
                        ot.append(t_)
                    for n in range(2):
                        y_ps = ps.tile([128, 1024], F32, name="pst", tag="pst")
                        for kc in range(NP):
                            nc.tensor.matmul(
                                y_ps[:, 0:512], lhsT=ot[kc],
                                rhs=wo_t[kc][:, 512 * n:512 * (n + 1)],
                                start=(kc == 0), stop=(kc == NP - 1))
                        ysb = oproj.tile([128, 512], F32, name="ysb",
                                         tag="ysb", bufs=3)
                        nc.vector.tensor_add(out=ysb, in0=y_ps[:, 0:512],
                                             in1=bo_sb[:, 512 * n:512 * (n + 1)])
                        nc.sync.dma_start(
                            out=yT[128 * mi:128 * (mi + 1),
                                   512 * n:512 * (n + 1)],
                            in_=ysb)

    nc.compile()
    return nc


_NC_CACHE = []


def kernel_with_results(x, wq, wk, wv, wo, bo, **run_kwargs):
    x = np.asarray(x, dtype=np.float32)
    wqT = np.ascontiguousarray(np.asarray(wq, dtype=np.float32).T)
    wkT = np.ascontiguousarray(np.asarray(wk, dtype=np.float32).T)
    wvT = np.ascontiguousarray(np.asarray(wv, dtype=np.float32).T)
    woT = np.ascontiguousarray(np.asarray(wo, dtype=np.float32).T)
    bo2 = np.asarray(bo, dtype=np.float32).reshape(1, D)

    if not _NC_CACHE:
        _NC_CACHE.append(build_nc())
    nc = _NC_CACHE[0]

    in_maps = []
    for c in range(NCORES):
        b, half = divmod(c, 2)
        xb = np.ascontiguousarray(x[b, :, 0, :])
        in_maps.append({
            "x": xb,
            "xq": np.ascontiguousarray(xb[:, half * LH:(half + 1) * LH]),
            "wqT": wqT, "wkT": wkT, "wvT": wvT, "woT": woT, "bo": bo2,
            "ones32": np.ones((1, 64), dtype=np.float32),
            "ones16": np.ones((1, NJC), dtype=np.float16),
        })

    kres = run_bass_kernel_spmd(nc, in_maps, list(range(NCORES)), **run_kwargs)

    out = np.empty((B, D, 1, L), dtype=np.float32)
    for c in range(NCORES):
        b, half = divmod(c, 2)
        out[b, :, 0, half * LH:(half + 1) * LH] = kres.results[c]["yT"].T
    return out, kres


def kernel(x, wq, wk, wv, wo, bo):
    out, _ = kernel_with_results(x, wq, wk, wv, wo, bo)
    return out
